# revision 68
# baseline (speedup 1.0000x reference)
"""DMGCGRUCell Trainium2 kernel: 8-core SPMD, pure batch sharding (2 batches/core).

Layout notes:
- Per core: BL=2 batches, full region rows S=1000 (padded 1024), full t (padded 1024).
- Passes fused per region r (regions independent): u/r blocks then candidate
  block reuse the same An tiles -> An read once per region.
- An host-normalized AND host-transposed to [G,R,t,s] bf16; deg from f32 A.
- Feature-major ("T") data tiles in bf16; PSUM accumulation f32.
- Attention batched over (b,s) = 2048 columns; softmax normalization applied
  after the alpha-weighted combine; sigmoid via exp+reciprocal so the scalar
  engine stays on one activation-table set (exp_and_others).
"""
import numpy as np
import concourse.bass as bass
import concourse.tile as tile
from concourse import bacc, mybir
from concourse.bass_utils import run_bass_kernel_spmd

B, N, R, S, G = 16, 10000, 10, 1000, 4
DIN, DH = 32, 64
NCORES = 8
BL = B // NCORES      # 2 local batches
SP = 1024             # padded s (output rows per region)
TP = 1024             # padded t (contraction dim)
NTC = TP // 128       # 8 t-chunks
NSC = SP // 512       # 2 s-halves of 512 (psum bank width)
W2 = BL * SP          # 2048 merged (b, s) columns
F32 = mybir.dt.float32
F32R = mybir.dt.float32r
BF16 = mybir.dt.bfloat16
AF = mybir.ActivationFunctionType
ALU = mybir.AluOpType
BIAS_W = np.array([0.1, 0.1, 0.1, 1.0], dtype=np.float32)

_cache = {}


def _build():
    nc = bacc.Bacc("TRN2", target_bir_lowering=False, debug=False, num_devices=NCORES)

    def din(name, shape, dt=BF16):
        return nc.dram_tensor(name, list(shape), dt, kind="ExternalInput").ap()

    xT = din("xT", (BL, R, 32, TP))
    hT = din("hT", (BL, R, 64, TP))
    AnT = din("AnT", (G, R, TP, SP))
    rsT = din("rsT", (R, 2, W2))
    Wur = din("Wur", (96, 512))          # cols (blk2, g4, e64)
    Wc = din("Wc", (96, 256))            # cols (g4, e64)
    a1w = {k: din(f"a1w_{k}", (258, 64)) for k in "urc"}
    a1b = {k: din(f"a1b_{k}", (64, 1), F32) for k in "urc"}
    a2wb = {k: din(f"a2wb_{k}", (65, 4)) for k in "urc"}
    a2b = {k: din(f"a2b_{k}", (4, 1), F32) for k in "urc"}
    identb = din("identb", (128, 128))
    o4 = din("o4", (4, 1))
    o164 = din("o164", (1, 64))
    o14 = din("o14", (1, 4))
    sel2 = din("sel2", (128, 64))
    sel4 = din("sel4", (4, 256))
    out_l = nc.dram_tensor("out_l", [BL, R, S, 64], F32, kind="ExternalOutput").ap()

    with tile.TileContext(nc, trace_sim=False) as tc:
        import contextlib
        ctx = contextlib.ExitStack()
        with ctx, nc.allow_low_precision(reason="bf16 data tiles; matmul accumulation in f32 PSUM"):
            sb = ctx.enter_context(tc.tile_pool(name="sb", bufs=1))
            sb2 = ctx.enter_context(tc.tile_pool(name="sb2", bufs=2))
            const = ctx.enter_context(tc.tile_pool(name="const", bufs=1))
            psA = ctx.enter_context(tc.tile_pool(name="psA", bufs=2, space="PSUM"))
            psB = ctx.enter_context(tc.tile_pool(name="psB", bufs=2, space="PSUM"))

            # ---- constants
            def cload(name, src, shape):
                t = const.tile(list(shape), BF16, tag=name, name=name)
                nc.sync.dma_start(t[:], src)
                return t

            id_t = cload("identb", identb[:], (128, 128))
            wur_t = cload("wur", Wur[:], (96, 512))
            wc_t = cload("wc", Wc[:], (96, 256))
            o4_t = cload("o4", o4[:], (4, 1))
            o164_t = cload("o164", o164[:], (1, 64))
            o14_t = cload("o14", o14[:], (1, 4))
            sel2_t = cload("sel2", sel2[:], (128, 64))
            sel_t = cload("sel4", sel4[:], (4, 256))
            a1w_t, a1b_t, a2wb_t, a2b_t = {}, {}, {}, {}
            for k in "urc":
                a1w_t[k] = []
                for ci, (r0, r1) in enumerate(((0, 128), (128, 256), (256, 258))):
                    a1w_t[k].append(cload(f"a1w{k}{ci}", a1w[k][r0:r1, :], (r1 - r0, 64)))
                b = const.tile([64, 1], F32, tag=f"a1b{k}", name=f"a1b{k}")
                nc.sync.dma_start(b[:], a1b[k][:])
                a1b_t[k] = b
                a2wb_t[k] = cload(f"a2wb{k}", a2wb[k][:], (65, 4))
                b2 = const.tile([4, 1], F32, tag=f"a2b{k}", name=f"a2b{k}")
                nc.sync.dma_start(b2[:], a2b[k][:])
                a2b_t[k] = b2

            def mm1(inp_tiles, w_tile, hw_tiles, ncols, nblk):
                """hw[tc][:, (blk,g,b,e)] = inp_b^T W ; b-interleaved copy (PSUM->SBUF)."""
                for tcd in range(NTC):
                    for b in range(BL):
                        ps = psA.tile([128, ncols], F32, tag="mm1")
                        nc.tensor.matmul(ps[:], inp_tiles[b][:, tcd * 128:(tcd + 1) * 128],
                                         w_tile[:], start=True, stop=True)
                        src = ps[:, :].rearrange("p (q e) -> p q e", e=64)
                        dst = hw_tiles[tcd][:, :].rearrange(
                            "p (q b e) -> p q b e", b=BL, e=64)[:, :, b]
                        if tcd % 2 == 0:
                            nc.vector.tensor_copy(dst, src)
                        else:
                            nc.scalar.copy(dst, src)

            def mm2(g, blk, ATd, hw_tiles, col0, HT, r):
                """GCN aggregate for graph g into HT[blk] tiles, relu'd."""
                pss = psB.tile([128, 2 * 512], F32, tag="mm2", name="mm2")
                for tcd in range(NTC):
                    lhs = hw_tiles[tcd][:, col0:col0 + 128]
                    for sc in range(NSC):
                        nc.tensor.matmul(pss[:, sc * 512:(sc + 1) * 512], lhs,
                                         ATd[tcd][:, sc * 512:(sc + 1) * 512],
                                         start=(tcd == 0), stop=(tcd == NTC - 1))
                lohi, off = g // 2, (g % 2) * 64
                for b in range(BL):
                    nc.scalar.activation(
                        HT[lohi][off:off + 64, b * SP:(b + 1) * SP],
                        pss[b * 64:(b + 1) * 64, :], AF.Relu)

            def att_core(blk, HT, rsT_t, rs1_t, zS):
                """z/logits/softmax-weights; returns (aU list of [4,512], rec [1,W2])."""
                nc.vector.tensor_scalar(zS[64:65, :], rs1_t[:], 0.5, None, op0=ALU.is_gt)
                for k in range(4):
                    cs = slice(k * 512, (k + 1) * 512)
                    zp = psA.tile([64, 512], F32, tag="att", name="zps")
                    nc.tensor.matmul(zp[:], a1w_t[blk][0][:], HT[0][:, cs], start=True, stop=False)
                    nc.tensor.matmul(zp[:], a1w_t[blk][1][:], HT[1][:, cs], start=False, stop=False)
                    nc.tensor.matmul(zp[:], a1w_t[blk][2][:], rsT_t[:, cs], start=False, stop=True)
                    nc.scalar.activation(zS[0:64, cs], zp[:], AF.Relu, bias=a1b_t[blk][:])
                # stage-grouped emission: each engine's queue stays unblocked
                # (in-order PE must not stall on a chunk whose act/DVE dep is late)
                rec = sb2.tile([1, W2], BF16, tag=f"rec{blk}", name=f"rec{blk}", bufs=1)
                lgs = []
                for k in range(4):
                    lg = psA.tile([4, 512], F32, tag="att", name="lg")
                    nc.tensor.matmul(lg[:], a2wb_t[blk][:],
                                     zS[:, k * 512:(k + 1) * 512], start=True, stop=True)
                    lgs.append(lg)
                aU = []
                for k in range(4):
                    aU_k = sb2.tile([4, 512], BF16, tag=f"aU{blk}{k}", name=f"aU{blk}{k}", bufs=1)
                    nc.scalar.activation(aU_k[:], lgs[k][:], AF.Exp, bias=a2b_t[blk][:])
                    aU.append(aU_k)
                sms = []
                for k in range(4):
                    sm = psA.tile([1, 512], F32, tag="att", name="sm")
                    nc.tensor.matmul(sm[:], o4_t[:], aU[k][:], start=True, stop=True)
                    sms.append(sm)
                for k in range(4):
                    nc.vector.reciprocal(rec[:, k * 512:(k + 1) * 512], sms[k][:])
                rb4s = []
                for k in range(4):
                    rb4 = psA.tile([4, 512], F32, tag="att", name="rb4")
                    nc.tensor.matmul(rb4[:], o14_t[:], rec[:, k * 512:(k + 1) * 512],
                                     start=True, stop=True)
                    rb4s.append(rb4)
                for k in range(4):
                    nc.vector.tensor_mul(aU[k][:], aU[k][:], rb4s[k][:])
                return aU, rec

            def att_combine(blk, HT, aU, rec, consume):
                """Per chunk: paired bcast of normalized alpha, [128,512] muls, PE
                pair-reduce into one PSUM tile; consume(k, cs, pacc) drains it."""
                for k in range(4):
                    cs = slice(k * 512, (k + 1) * 512)
                    ms = []
                    for p in range(2):
                        ab = psA.tile([128, 512], F32, tag="att", name="ab")
                        nc.tensor.matmul(ab[:], sel_t[:, p * 128:(p + 1) * 128],
                                         aU[k][:], start=True, stop=True)
                        m = sb2.tile([128, 512], BF16, tag=f"cm{p}", name=f"cm{p}", bufs=2)
                        nc.vector.tensor_mul(m[:], HT[p][:, cs], ab[:])
                        ms.append(m)
                    pacc = psA.tile([64, 512], F32, tag="att", name="pacc")
                    nc.tensor.matmul(pacc[:], sel2_t[:], ms[0][:], start=True, stop=False)
                    nc.tensor.matmul(pacc[:], sel2_t[:], ms[1][:], start=False, stop=True)
                    consume(k, cs, pacc)

            def gru_out(uT, th, inpT1, r):
                """o = hp + u*(th - hp); transpose to [s, e]; store."""
                for b in range(BL):
                    bs = slice(b * SP, (b + 1) * SP)
                    hp = inpT1[b][0:64, :]
                    d = sb2.tile([64, SP], BF16, tag=f"d{b}", name=f"d{b}", bufs=1)
                    nc.vector.tensor_sub(d[:], th[:, bs], hp)
                    nc.vector.tensor_mul(d[:], uT[:, bs], d[:])
                    nc.vector.tensor_add(d[:], d[:], hp)
                    tp = psA.tile([128, 512], BF16, tag="att", name="tps")
                    for ci in range(8):
                        nc.tensor.transpose(tp[:, ci * 64:(ci + 1) * 64],
                                            d[:, ci * 128:(ci + 1) * 128], id_t[0:64, 0:64])
                    gs = sb2.tile([128, 512], F32, tag=f"gs{b}", name=f"gs{b}", bufs=1)
                    if b == 0:
                        nc.vector.tensor_copy(gs[:], tp[:])
                    else:
                        nc.scalar.copy(gs[:], tp[:])
                    dst1 = out_l[b, r, 0:896, :].rearrange("(c p) e -> p c e", p=128)
                    src1 = gs[:, 0:448].rearrange("p (c e) -> p c e", c=7)
                    nc.sync.dma_start(dst1, src1)
                    nc.sync.dma_start(out_l[b, r, 896:1000, :], gs[0:104, 448:512])

            # ================= fused region loop =================
            prev_gru = None
            for r in range(R):
                # -- input tiles (double-buffered; DMAs overlap previous region)
                inpT1, inpT2 = [], []
                for b in range(BL):
                    t1 = sb2.tile([96, TP], BF16, tag=f"inp1{b}", name=f"inp1{b}")
                    nc.sync.dma_start(t1[0:64, :], hT[b, r])
                    nc.sync.dma_start(t1[64:96, :], xT[b, r])
                    inpT1.append(t1)
                    t2 = sb2.tile([96, TP], BF16, tag=f"inp2{b}", name=f"inp2{b}")
                    nc.sync.dma_start(t2[64:96, :], xT[b, r])
                    inpT2.append(t2)
                rsT_t = sb2.tile([2, W2], BF16, tag="rsT", name="rsT", bufs=1)
                nc.sync.dma_start(rsT_t[:], rsT[r])
                rs1_t = sb2.tile([1, W2], BF16, tag="rs1", name="rs1", bufs=1)
                nc.sync.dma_start(rs1_t[:], rsT[r, 1:2, :])
                ATd = {g: [] for g in range(G)}
                for g in range(G):
                    for tcd in range(NTC):
                        a = sb2.tile([128, SP], BF16, tag=f"at{g}{tcd}", name=f"at{g}{tcd}", bufs=1)
                        nc.sync.dma_start(a[:], AnT[g, r, tcd * 128:(tcd + 1) * 128, :])
                        ATd[g].append(a)

                # -- pass 1: blocks u, r
                hw1 = [sb.tile([128, 1024], BF16, tag=f"hw1{tcd}", name=f"hw1{tcd}")
                       for tcd in range(NTC)]
                mm1(inpT1, wur_t, hw1, 512, 2)
                # previous region's GRU/store: PE transposes slot in here while
                # this region's mm2 deps (copies) drain
                if prev_gru is not None:
                    gru_out(*prev_gru)
                HT = {blk: [sb2.tile([128, W2], BF16, tag=f"HT{blk}{lh}", name=f"HT{blk}{lh}", bufs=1)
                            for lh in range(2)] for blk in ("u", "r")}
                for g in range(G):
                    for bi, blk in enumerate(("u", "r")):
                        mm2(g, blk, ATd[g], hw1, bi * 512 + g * 128, HT[blk], r)

                zSu = sb2.tile([65, W2], BF16, tag="zSu", name="zSu", bufs=1)
                aUu, recu = att_core("u", HT["u"], rsT_t, rs1_t, zSu)
                zSr = sb2.tile([65, W2], BF16, tag="zSr", name="zSr", bufs=1)
                aUr, recr = att_core("r", HT["r"], rsT_t, rs1_t, zSr)

                # r-combine first: rh feeds candidate mm1 (PE does u-combine bcasts meanwhile)
                def consume_r(k, cs, pacc):
                    # chunk k covers batch k//2, local s-half k%2: rh = alpha.H * hp
                    b, sh = k // 2, k % 2
                    hs = slice(sh * 512, (sh + 1) * 512)
                    nc.vector.tensor_mul(inpT2[b][0:64, hs], pacc[:],
                                         inpT1[b][0:64, hs])
                att_combine("r", HT["r"], aUr, recr, consume_r)

                eU = sb2.tile([64, W2], BF16, tag="eU", name="eU", bufs=1)
                # u = sigmoid(acc) = 1/(1+exp(-acc))
                def consume_u(k, cs, pacc):
                    nc.scalar.activation(eU[:, cs], pacc[:], AF.Exp, scale=-1.0)
                att_combine("u", HT["u"], aUu, recu, consume_u)
                nc.gpsimd.tensor_scalar_add(eU[:], eU[:], 1.0)
                uT = sb2.tile([64, W2], BF16, tag="uT", name="uT", bufs=1)
                nc.vector.reciprocal(uT[:], eU[:])

                # -- pass 2: candidate block (reuses ATd tiles)
                hw2 = [sb.tile([128, 512], BF16, tag=f"hw2{tcd}", name=f"hw2{tcd}")
                       for tcd in range(NTC)]
                mm1(inpT2, wc_t, hw2, 256, 1)
                HTc = [sb2.tile([128, W2], BF16, tag=f"HTc{lh}", name=f"HTc{lh}", bufs=1)
                       for lh in range(2)]
                for g in range(G):
                    mm2(g, "c", ATd[g], hw2, g * 128, HTc, r)
                zSc = sb2.tile([65, W2], BF16, tag="zSc", name="zSc", bufs=1)
                aUc, recc = att_core("c", HTc, rsT_t, rs1_t, zSc)
                th = sb2.tile([64, W2], BF16, tag="th", name="th", bufs=1)
                def consume_c(k, cs, pacc):
                    nc.scalar.activation(th[:, cs], pacc[:], AF.Tanh)
                att_combine("c", HTc, aUc, recc, consume_c)
                prev_gru = (uT, th, inpT1, r)
            gru_out(*prev_gru)

    nc.compile()
    return nc


def _prep(inputs):
    """Host-side shard + layout prep. Returns in_maps (len 8)."""
    import ml_dtypes
    bf = ml_dtypes.bfloat16

    A = np.asarray(inputs["A"], np.float32)
    deg = np.clip(A.sum(-1), 1e-5, None) ** -0.5          # [G,R,S]
    An_f = deg[..., :, None] * (A + np.eye(S, dtype=np.float32)) * deg[..., None, :]
    AnT = np.zeros((G, R, TP, SP), bf)
    AnT[:, :, 0:S, 0:S] = An_f.transpose(0, 1, 3, 2)       # [t, s]

    x_t = np.asarray(inputs["x_t"], np.float32).reshape(B, R, S, DIN)
    h_prev = np.asarray(inputs["h_prev"], np.float32).reshape(B, R, S, DH)
    rs = np.asarray(inputs["resid_stats"], np.float32).reshape(B, R, S, 2)
    xT_f = np.zeros((B, R, 32, TP), bf)
    xT_f[..., 0:S] = x_t.transpose(0, 1, 3, 2)
    hT_f = np.zeros((B, R, 64, TP), bf)
    hT_f[..., 0:S] = h_prev.transpose(0, 1, 3, 2)

    perm = np.concatenate([np.arange(32, 96), np.arange(0, 32)])  # rows -> (h, x)
    Wur = np.concatenate([inputs["W_u"].transpose(1, 0, 2).reshape(96, 256),
                          inputs["W_r"].transpose(1, 0, 2).reshape(96, 256)], axis=1)[perm]
    Wc = np.ascontiguousarray(inputs["W_c"].transpose(1, 0, 2).reshape(96, 256)[perm])
    log1p_bw = np.log1p(BIAS_W).reshape(1, 4)
    common = {
        "Wur": np.ascontiguousarray(Wur).astype(bf),
        "Wc": Wc.astype(bf),
        "identb": np.eye(128, dtype=bf),
        "o4": np.ones((4, 1), bf),
        "o164": np.ones((1, 64), bf),
        "o14": np.ones((1, 4), bf),
        "sel2": np.concatenate([np.eye(64, dtype=np.float32)] * 2, axis=0).astype(bf),
        "sel4": np.kron(np.eye(4, dtype=np.float32), np.ones((1, 64), np.float32)).astype(bf),
    }
    for k in "urc":
        common[f"a1w_{k}"] = np.asarray(inputs[f"a1w_{k}"], np.float32).astype(bf)
        common[f"a1b_{k}"] = np.asarray(inputs[f"a1b_{k}"], np.float32).reshape(64, 1)
        common[f"a2wb_{k}"] = np.concatenate(
            [np.asarray(inputs[f"a2w_{k}"], np.float32), log1p_bw], axis=0).astype(bf)
        common[f"a2b_{k}"] = np.asarray(inputs[f"a2b_{k}"], np.float32).reshape(4, 1)

    in_maps = []
    for core in range(NCORES):
        bs = slice(core * BL, (core + 1) * BL)
        rsT_c = np.zeros((R, 2, W2), bf)
        for b in range(BL):
            rsT_c[:, :, b * SP:b * SP + S] = rs[core * BL + b].transpose(0, 2, 1)
        m = dict(common)
        m["xT"] = np.ascontiguousarray(xT_f[bs])
        m["hT"] = np.ascontiguousarray(hT_f[bs])
        m["AnT"] = AnT
        m["rsT"] = rsT_c
        in_maps.append(m)
    return in_maps


def kernel(**inputs) -> np.ndarray:
    if "nc" not in _cache:
        _cache["nc"] = _build()
    nc = _cache["nc"]
    in_maps = _prep(inputs)
    res = run_bass_kernel_spmd(nc, in_maps, list(range(NCORES)))
    out = np.zeros((B, R, S, DH), np.float32)
    for core in range(NCORES):
        out[core * BL:(core + 1) * BL] = res.results[core]["out_l"]
    return out.reshape(B, N, DH)


# revision 71
# speedup vs baseline: 1.2725x; 1.2725x over previous
"""DMGCGRUCell Trainium2 kernel: 8-core SPMD, pure batch sharding (2 batches/core).

Layout notes:
- Per core: BL=2 batches, full region rows S=1000 (padded 1024), full t (padded 1024).
- Passes fused per region r (regions independent): u/r blocks then candidate
  block reuse the same An tiles -> An read once per region.
- An host-normalized AND host-transposed to [G,R,t,s] bf16; deg from f32 A.
- Feature-major ("T") data tiles in bf16; PSUM accumulation f32.
- Attention batched over (b,s) = 2048 columns; softmax normalization applied
  after the alpha-weighted combine; sigmoid via exp+reciprocal so the scalar
  engine stays on one activation-table set (exp_and_others).
"""
import numpy as np
import concourse.bass as bass
import concourse.tile as tile
from concourse import bacc, mybir
from concourse.bass_utils import run_bass_kernel_spmd

B, N, R, S, G = 16, 10000, 10, 1000, 4
DIN, DH = 32, 64
NCORES = 8
BL = B // NCORES      # 2 local batches
SP = 1024             # padded s (output rows per region)
TP = 1024             # padded t (contraction dim)
NTC = TP // 128       # 8 t-chunks
NSC = SP // 512       # 2 s-halves of 512 (psum bank width)
W2 = BL * SP          # 2048 merged (b, s) columns
F32 = mybir.dt.float32
F32R = mybir.dt.float32r
BF16 = mybir.dt.bfloat16
AF = mybir.ActivationFunctionType
ALU = mybir.AluOpType
BIAS_W = np.array([0.1, 0.1, 0.1, 1.0], dtype=np.float32)

_cache = {}


def _build():
    nc = bacc.Bacc("TRN2", target_bir_lowering=False, debug=False, num_devices=NCORES)

    def din(name, shape, dt=BF16):
        return nc.dram_tensor(name, list(shape), dt, kind="ExternalInput").ap()

    xT = din("xT", (BL, R, 32, TP))
    hT = din("hT", (BL, R, 64, TP))
    AnT = din("AnT", (G, R, TP, SP))
    rsT = din("rsT", (R, 2, W2))
    Wur = din("Wur", (96, 512))          # cols (blk2, g4, e64)
    Wc = din("Wc", (96, 256))            # cols (g4, e64)
    a1w = {k: din(f"a1w_{k}", (258, 64)) for k in "urc"}
    a1b = {k: din(f"a1b_{k}", (64, 1), F32) for k in "urc"}
    a2wb = {k: din(f"a2wb_{k}", (65, 4)) for k in "urc"}
    a2b = {k: din(f"a2b_{k}", (4, 1), F32) for k in "urc"}
    identb = din("identb", (128, 128))
    o4 = din("o4", (4, 1))
    o164 = din("o164", (1, 64))
    o14 = din("o14", (1, 4))
    sel2 = din("sel2", (128, 64))
    sel4 = din("sel4", (4, 256))
    out_l = nc.dram_tensor("out_l", [BL, R, S, 64], F32, kind="ExternalOutput").ap()

    with tile.TileContext(nc, trace_sim=False) as tc:
        import contextlib
        ctx = contextlib.ExitStack()
        with ctx, nc.allow_low_precision(reason="bf16 data tiles; matmul accumulation in f32 PSUM"):
            sb = ctx.enter_context(tc.tile_pool(name="sb", bufs=1))
            sb2 = ctx.enter_context(tc.tile_pool(name="sb2", bufs=2))
            const = ctx.enter_context(tc.tile_pool(name="const", bufs=1))
            psA = ctx.enter_context(tc.tile_pool(name="psA", bufs=2, space="PSUM"))
            psB = ctx.enter_context(tc.tile_pool(name="psB", bufs=2, space="PSUM"))

            # ---- constants
            def cload(name, src, shape):
                t = const.tile(list(shape), BF16, tag=name, name=name)
                nc.sync.dma_start(t[:], src)
                return t

            id_t = cload("identb", identb[:], (128, 128))
            wur_t = cload("wur", Wur[:], (96, 512))
            wc_t = cload("wc", Wc[:], (96, 256))
            o4_t = cload("o4", o4[:], (4, 1))
            o164_t = cload("o164", o164[:], (1, 64))
            o14_t = cload("o14", o14[:], (1, 4))
            sel2_t = cload("sel2", sel2[:], (128, 64))
            sel_t = cload("sel4", sel4[:], (4, 256))
            a1w_t, a1b_t, a2wb_t, a2b_t = {}, {}, {}, {}
            for k in "urc":
                a1w_t[k] = []
                for ci, (r0, r1) in enumerate(((0, 128), (128, 256), (256, 258))):
                    a1w_t[k].append(cload(f"a1w{k}{ci}", a1w[k][r0:r1, :], (r1 - r0, 64)))
                b = const.tile([64, 1], F32, tag=f"a1b{k}", name=f"a1b{k}")
                nc.sync.dma_start(b[:], a1b[k][:])
                a1b_t[k] = b
                a2wb_t[k] = cload(f"a2wb{k}", a2wb[k][:], (65, 4))
                b2 = const.tile([4, 1], F32, tag=f"a2b{k}", name=f"a2b{k}")
                nc.sync.dma_start(b2[:], a2b[k][:])
                a2b_t[k] = b2

            def mm1(inp_tiles, w_tile, hw_tiles, ncols, nblk):
                """hw[tc][:, (blk,g,b,e)] = inp_b^T W ; b-interleaved copy (PSUM->SBUF)."""
                for tcd in range(NTC):
                    for b in range(BL):
                        ps = psA.tile([128, ncols], F32, tag="mm1")
                        nc.tensor.matmul(ps[:], inp_tiles[b][:, tcd * 128:(tcd + 1) * 128],
                                         w_tile[:], start=True, stop=True)
                        src = ps[:, :].rearrange("p (q e) -> p q e", e=64)
                        dst = hw_tiles[tcd][:, :].rearrange(
                            "p (q b e) -> p q b e", b=BL, e=64)[:, :, b]
                        if tcd % 2 == 0:
                            nc.vector.tensor_copy(dst, src)
                        else:
                            nc.scalar.copy(dst, src)

            def mm2(g, blk, ATd, hw_tiles, col0, HT, r):
                """GCN aggregate for graph g into HT[blk] tiles, relu'd."""
                pss = psB.tile([128, 2 * 512], F32, tag="mm2", name="mm2")
                for tcd in range(NTC):
                    lhs = hw_tiles[tcd][:, col0:col0 + 128]
                    for sc in range(NSC):
                        nc.tensor.matmul(pss[:, sc * 512:(sc + 1) * 512], lhs,
                                         ATd[tcd][:, sc * 512:(sc + 1) * 512],
                                         start=(tcd == 0), stop=(tcd == NTC - 1))
                lohi, off = g // 2, (g % 2) * 64
                for b in range(BL):
                    nc.scalar.activation(
                        HT[lohi][off:off + 64, b * SP:(b + 1) * SP],
                        pss[b * 64:(b + 1) * 64, :], AF.Relu)

            def att_core(blk, HT, rsT_t, rs1_t, zS):
                """z/logits/softmax-weights; returns (aU list of [4,512], rec [1,W2])."""
                nc.vector.tensor_scalar(zS[64:65, :], rs1_t[:], 0.5, None, op0=ALU.is_gt)
                for k in range(4):
                    cs = slice(k * 512, (k + 1) * 512)
                    zp = psA.tile([64, 512], F32, tag="att", name="zps")
                    nc.tensor.matmul(zp[:], a1w_t[blk][0][:], HT[0][:, cs], start=True, stop=False)
                    nc.tensor.matmul(zp[:], a1w_t[blk][1][:], HT[1][:, cs], start=False, stop=False)
                    nc.tensor.matmul(zp[:], a1w_t[blk][2][:], rsT_t[:, cs], start=False, stop=True)
                    nc.scalar.activation(zS[0:64, cs], zp[:], AF.Relu, bias=a1b_t[blk][:])
                # stage-grouped emission: each engine's queue stays unblocked
                # (in-order PE must not stall on a chunk whose act/DVE dep is late)
                rec = sb2.tile([1, W2], BF16, tag=f"rec{blk}", name=f"rec{blk}", bufs=1)
                lgs = []
                for k in range(4):
                    lg = psA.tile([4, 512], F32, tag="att", name="lg")
                    nc.tensor.matmul(lg[:], a2wb_t[blk][:],
                                     zS[:, k * 512:(k + 1) * 512], start=True, stop=True)
                    lgs.append(lg)
                aU = []
                for k in range(4):
                    aU_k = sb2.tile([4, 512], BF16, tag=f"aU{blk}{k}", name=f"aU{blk}{k}", bufs=1)
                    nc.scalar.activation(aU_k[:], lgs[k][:], AF.Exp, bias=a2b_t[blk][:])
                    aU.append(aU_k)
                sms = []
                for k in range(4):
                    sm = psA.tile([1, 512], F32, tag="att", name="sm")
                    nc.tensor.matmul(sm[:], o4_t[:], aU[k][:], start=True, stop=True)
                    sms.append(sm)
                for k in range(4):
                    nc.vector.reciprocal(rec[:, k * 512:(k + 1) * 512], sms[k][:])
                rb4s = []
                for k in range(4):
                    rb4 = psA.tile([4, 512], F32, tag="att", name="rb4")
                    nc.tensor.matmul(rb4[:], o14_t[:], rec[:, k * 512:(k + 1) * 512],
                                     start=True, stop=True)
                    rb4s.append(rb4)
                for k in range(4):
                    nc.vector.tensor_mul(aU[k][:], aU[k][:], rb4s[k][:])
                return aU, rec

            def att_combine(blk, HT, aU, rec, consume):
                """Per chunk: paired bcast of normalized alpha, [128,512] muls, PE
                pair-reduce into one PSUM tile; consume(k, cs, pacc) drains it."""
                for k in range(4):
                    cs = slice(k * 512, (k + 1) * 512)
                    ms = []
                    for p in range(2):
                        ab = psA.tile([128, 512], F32, tag="att", name="ab")
                        nc.tensor.matmul(ab[:], sel_t[:, p * 128:(p + 1) * 128],
                                         aU[k][:], start=True, stop=True)
                        m = sb2.tile([128, 512], BF16, tag=f"cm{p}", name=f"cm{p}", bufs=2)
                        nc.vector.tensor_mul(m[:], HT[p][:, cs], ab[:])
                        ms.append(m)
                    pacc = psA.tile([64, 512], F32, tag="att", name="pacc")
                    nc.tensor.matmul(pacc[:], sel2_t[:], ms[0][:], start=True, stop=False)
                    nc.tensor.matmul(pacc[:], sel2_t[:], ms[1][:], start=False, stop=True)
                    consume(k, cs, pacc)

            def gru_out(uT, th, inpT1, r):
                """o = hp + u*(th - hp); transpose to [s, e]; store."""
                for b in range(BL):
                    bs = slice(b * SP, (b + 1) * SP)
                    hp = inpT1[b][0:64, :]
                    d = sb2.tile([64, SP], BF16, tag=f"d{b}", name=f"d{b}", bufs=1)
                    nc.vector.tensor_sub(d[:], th[:, bs], hp)
                    nc.vector.tensor_mul(d[:], uT[:, bs], d[:])
                    nc.vector.tensor_add(d[:], d[:], hp)
                    tp = psA.tile([128, 512], BF16, tag="att", name="tps")
                    for ci in range(8):
                        nc.tensor.transpose(tp[:, ci * 64:(ci + 1) * 64],
                                            d[:, ci * 128:(ci + 1) * 128], id_t[0:64, 0:64])
                    gs = sb2.tile([128, 512], F32, tag=f"gs{b}", name=f"gs{b}", bufs=1)
                    if b == 0:
                        nc.vector.tensor_copy(gs[:], tp[:])
                    else:
                        nc.scalar.copy(gs[:], tp[:])
                    dst1 = out_l[b, r, 0:896, :].rearrange("(c p) e -> p c e", p=128)
                    src1 = gs[:, 0:448].rearrange("p (c e) -> p c e", c=7)
                    nc.sync.dma_start(dst1, src1)
                    nc.sync.dma_start(out_l[b, r, 896:1000, :], gs[0:104, 448:512])

            # ================= fused region loop =================
            prev_gru = None
            for r in range(R):
                # -- input tiles (double-buffered; DMAs overlap previous region)
                inpT1, inpT2 = [], []
                for b in range(BL):
                    t1 = sb2.tile([96, TP], BF16, tag=f"inp1{b}", name=f"inp1{b}")
                    nc.sync.dma_start(t1[0:64, :], hT[b, r])
                    nc.sync.dma_start(t1[64:96, :], xT[b, r])
                    inpT1.append(t1)
                    t2 = sb2.tile([96, TP], BF16, tag=f"inp2{b}", name=f"inp2{b}")
                    nc.sync.dma_start(t2[64:96, :], xT[b, r])
                    inpT2.append(t2)
                rsT_t = sb2.tile([2, W2], BF16, tag="rsT", name="rsT", bufs=1)
                nc.sync.dma_start(rsT_t[:], rsT[r])
                rs1_t = sb2.tile([1, W2], BF16, tag="rs1", name="rs1", bufs=1)
                nc.sync.dma_start(rs1_t[:], rsT[r, 1:2, :])
                ATd = {g: [] for g in range(G)}
                for g in range(G):
                    for tcd in range(NTC):
                        a = sb2.tile([128, SP], BF16, tag=f"at{g}{tcd}", name=f"at{g}{tcd}", bufs=1)
                        nc.sync.dma_start(a[:], AnT[g, r, tcd * 128:(tcd + 1) * 128, :])
                        ATd[g].append(a)

                # -- pass 1: blocks u, r
                hw1 = [sb.tile([128, 1024], BF16, tag=f"hw1{tcd}", name=f"hw1{tcd}")
                       for tcd in range(NTC)]
                mm1(inpT1, wur_t, hw1, 512, 2)
                # previous region's GRU/store: PE transposes slot in here while
                # this region's mm2 deps (copies) drain
                if prev_gru is not None:
                    gru_out(*prev_gru)
                HT = {blk: [sb2.tile([128, W2], BF16, tag=f"HT{blk}{lh}", name=f"HT{blk}{lh}", bufs=1)
                            for lh in range(2)] for blk in ("u", "r")}
                for g in range(G):
                    for bi, blk in enumerate(("u", "r")):
                        mm2(g, blk, ATd[g], hw1, bi * 512 + g * 128, HT[blk], r)

                zSu = sb2.tile([65, W2], BF16, tag="zSu", name="zSu", bufs=1)
                aUu, recu = att_core("u", HT["u"], rsT_t, rs1_t, zSu)
                zSr = sb2.tile([65, W2], BF16, tag="zSr", name="zSr", bufs=1)
                aUr, recr = att_core("r", HT["r"], rsT_t, rs1_t, zSr)

                # r-combine first: rh feeds candidate mm1 (PE does u-combine bcasts meanwhile)
                def consume_r(k, cs, pacc):
                    # chunk k covers batch k//2, local s-half k%2: rh = alpha.H * hp
                    b, sh = k // 2, k % 2
                    hs = slice(sh * 512, (sh + 1) * 512)
                    nc.vector.tensor_mul(inpT2[b][0:64, hs], pacc[:],
                                         inpT1[b][0:64, hs])
                att_combine("r", HT["r"], aUr, recr, consume_r)

                eU = sb2.tile([64, W2], BF16, tag="eU", name="eU", bufs=1)
                # u = sigmoid(acc) = 1/(1+exp(-acc))
                def consume_u(k, cs, pacc):
                    nc.scalar.activation(eU[:, cs], pacc[:], AF.Exp, scale=-1.0)
                att_combine("u", HT["u"], aUu, recu, consume_u)
                nc.gpsimd.tensor_scalar_add(eU[:], eU[:], 1.0)
                uT = sb2.tile([64, W2], BF16, tag="uT", name="uT", bufs=1)
                nc.vector.reciprocal(uT[:], eU[:])

                # -- pass 2: candidate block (reuses ATd tiles)
                hw2 = [sb.tile([128, 512], BF16, tag=f"hw2{tcd}", name=f"hw2{tcd}")
                       for tcd in range(NTC)]
                mm1(inpT2, wc_t, hw2, 256, 1)
                HTc = [sb2.tile([128, W2], BF16, tag=f"HTc{lh}", name=f"HTc{lh}", bufs=1)
                       for lh in range(2)]
                for g in range(G):
                    mm2(g, "c", ATd[g], hw2, g * 128, HTc, r)
                zSc = sb2.tile([65, W2], BF16, tag="zSc", name="zSc", bufs=1)
                aUc, recc = att_core("c", HTc, rsT_t, rs1_t, zSc)
                th = sb2.tile([64, W2], BF16, tag="th", name="th", bufs=1)
                def consume_c(k, cs, pacc):
                    nc.scalar.activation(th[:, cs], pacc[:], AF.Tanh)
                att_combine("c", HTc, aUc, recc, consume_c)
                prev_gru = (uT, th, inpT1, r)
            gru_out(*prev_gru)

    nc.compile()
    return nc


def _prep(inputs):
    """Host-side shard + layout prep. Returns in_maps (len 8)."""
    import ml_dtypes
    bf = ml_dtypes.bfloat16

    A = np.asarray(inputs["A"], np.float32)
    deg = np.clip(A.sum(-1), 1e-5, None) ** -0.5          # [G,R,S]
    An_f = deg[..., :, None] * (A + np.eye(S, dtype=np.float32)) * deg[..., None, :]
    AnT = np.zeros((G, R, TP, SP), bf)
    AnT[:, :, 0:S, 0:S] = An_f.transpose(0, 1, 3, 2)       # [t, s]

    x_t = np.asarray(inputs["x_t"], np.float32).reshape(B, R, S, DIN)
    h_prev = np.asarray(inputs["h_prev"], np.float32).reshape(B, R, S, DH)
    rs = np.asarray(inputs["resid_stats"], np.float32).reshape(B, R, S, 2)
    xT_f = np.zeros((B, R, 32, TP), bf)
    xT_f[..., 0:S] = x_t.transpose(0, 1, 3, 2)
    hT_f = np.zeros((B, R, 64, TP), bf)
    hT_f[..., 0:S] = h_prev.transpose(0, 1, 3, 2)

    perm = np.concatenate([np.arange(32, 96), np.arange(0, 32)])  # rows -> (h, x)
    Wur = np.concatenate([inputs["W_u"].transpose(1, 0, 2).reshape(96, 256),
                          inputs["W_r"].transpose(1, 0, 2).reshape(96, 256)], axis=1)[perm]
    Wc = np.ascontiguousarray(inputs["W_c"].transpose(1, 0, 2).reshape(96, 256)[perm])
    log1p_bw = np.log1p(BIAS_W).reshape(1, 4)
    common = {
        "Wur": np.ascontiguousarray(Wur).astype(bf),
        "Wc": Wc.astype(bf),
        "identb": np.eye(128, dtype=bf),
        "o4": np.ones((4, 1), bf),
        "o164": np.ones((1, 64), bf),
        "o14": np.ones((1, 4), bf),
        "sel2": np.concatenate([np.eye(64, dtype=np.float32)] * 2, axis=0).astype(bf),
        "sel4": np.kron(np.eye(4, dtype=np.float32), np.ones((1, 64), np.float32)).astype(bf),
    }
    for k in "urc":
        common[f"a1w_{k}"] = np.asarray(inputs[f"a1w_{k}"], np.float32).astype(bf)
        common[f"a1b_{k}"] = np.asarray(inputs[f"a1b_{k}"], np.float32).reshape(64, 1)
        common[f"a2wb_{k}"] = np.concatenate(
            [np.asarray(inputs[f"a2w_{k}"], np.float32), log1p_bw], axis=0).astype(bf)
        common[f"a2b_{k}"] = np.asarray(inputs[f"a2b_{k}"], np.float32).reshape(4, 1)

    in_maps = []
    for core in range(NCORES):
        bs = slice(core * BL, (core + 1) * BL)
        rsT_c = np.zeros((R, 2, W2), bf)
        for b in range(BL):
            rsT_c[:, :, b * SP:b * SP + S] = rs[core * BL + b].transpose(0, 2, 1)
        m = dict(common)
        m["xT"] = np.ascontiguousarray(xT_f[bs])
        m["hT"] = np.ascontiguousarray(hT_f[bs])
        m["AnT"] = AnT
        m["rsT"] = rsT_c
        in_maps.append(m)
    return in_maps


def kernel(**inputs) -> np.ndarray:
    if "nc" not in _cache:
        _cache["nc"] = _build()
    nc = _cache["nc"]
    in_maps = _prep(inputs)
    res = run_bass_kernel_spmd(nc, in_maps, list(range(NCORES)))
    out = np.zeros((B, R, S, DH), np.float32)
    for core in range(NCORES):
        out[core * BL:(core + 1) * BL] = res.results[core]["out_l"]
    return out.reshape(B, N, DH)
